# revision 34
# baseline (speedup 1.0000x reference)
"""Trainium2 Bass kernel for nn_NoiseReduceLayer (noisereduce non-stationary
spectral gating): STFT -> |S| -> EMA filtfilt (time) -> sigmoid gate ->
separable (65x3) smoothing conv -> mask*S -> ISTFT.

Formulation (validated against the jax reference in a numpy prototype):
  - STFT/ISTFT as DFT matmuls with the hann window folded into the matrices;
    the 1/win.sum() forward scale and win.sum() inverse scale cancel and are
    both omitted (the gate ratio is scale-invariant).
  - Real-signal symmetry fold: u[n] = x[n]+x[2048-n], v[n] = x[n]-x[2048-n]
    (n = 1..1024) halve the DFT contraction; w[0] = 0 kills the n = 0 term.
  - Framing via 512-sample chunk matrices: xc (aligned) and xc1 (shifted by
    +1 sample) built by PE transposes of row-chunks; the x[2048-n] flip is
    one J-perm matmul per a-tile, shared by u (DVE add) and v (Pool sub).
  - Si tile 8 (rows 1024..1151) is identically zero (sin(pi*n) = 0): its
    forward chain, |S| square, and the inverse Q-contraction over it are
    skipped.
  - EMA filtfilt = dense (321x321) matrix M = P@L@P@L precomputed on host;
    reciprocal via the single-instruction DVE reciprocal_approx_fast.
  - 65x3 smoothing conv separable: 3-tap time conv on DVE/Pool + banded
    matmul over frequency.
  - Inverse: P = Br"@Sdr (9 k-tiles), Q = Bi"@Sdi (8 k-tiles), yp = P+Q,
    ym = 2*P-yp; combine perm-matmuls accumulate the overlap-add directly
    into pre-zeroed PSUM tiles (start=False, skip_group_check), then one
    normalize multiply and fp16 transposes feed the output DMA.

Scheduling (engine queues execute in emission order, so emission IS the
schedule): phase A runs a two-deep software pipeline -- head1(s) =
framing/fold/forward-DFT keeps the PE saturated while head2(s-1) =
|S|-transpose/EMA/ratio and tail(s-2) = sigmoid/conv/S*mask run on
ACT/DVE/Pool underneath. Phase B is sample-outer so each sample's
combine/store overlaps the next sample's inverse chains. PSUM evacuation
is spread across ACT (copies) and DVE (adds); Pool takes only SBUF-SBUF
ops (hardware: GPSIMD cannot access PSUM, engines read at most one PSUM
operand). Inverse weights stream on the sync+gpsimd DMA queues; the first
pairs preload during phase A.

Sharding: pure data parallel, batch 32 -> 4 samples on each of 8 cores.

Precision: all matmul inputs fp16 (fp32 PSUM accumulate). Measured error
vs the fp32 jax reference: ~1.1e-3 of output absmax. (fp8/DoubleRow was
prototyped and rejected: e4m3 mantissa noise puts output error at or
above the 2e-2 gate on every matmul stage, even with scaling.)
"""
import numpy as np

import concourse.mybir as mybir
from concourse import bacc
from concourse.tile import TileContext
from concourse.bass_utils import run_bass_kernel_spmd

f32 = mybir.dt.float32
f16 = mybir.dt.float16
AF = mybir.ActivationFunctionType
OP = mybir.AluOpType

SR = 16000
NFFT = 2048
HOP = 512
NSAMP = 163840
T = 321        # stft frames
F = 1025       # one-sided bins
FP = 1152      # padded bins = 9*128
NCH = 324      # 512-chunks in padded signal (2 zero + 320 data + 2 zero)
BLOC = 4       # samples per core
NCORES = 8

_TF = 2.0 * SR / HOP
B_EMA = (np.sqrt(1.0 + 4.0 * _TF**2) - 1.0) / (2.0 * _TF**2)


def _build_consts():
    w = 0.5 - 0.5 * np.cos(2 * np.pi * np.arange(NFFT) / NFFT)
    k = np.arange(FP)
    n = np.arange(1, 1025)                    # half range (w[0]=0 kills n=0)
    ang = 2 * np.pi * np.outer(k, n) / NFFT   # (FP, 1024)
    half = np.ones(1024)
    half[-1] = 0.5                            # n=1024 self-pairs in u
    CrH = np.cos(ang) * w[None, 1:1025] * half[None, :]
    CiH = -np.sin(ang) * w[None, 1:1025]
    CrH[F:, :] = 0.0
    CiH[F:, :] = 0.0
    ck = np.ones(FP)
    ck[0] = 0.5
    ck[1024] = 0.5
    ck[F:] = 0.0
    BrH = (2.0 / NFFT) * np.cos(ang) * ck[:, None] * w[None, 1:1025]   # (FP, 1024)
    BiH = -(2.0 / NFFT) * np.sin(ang) * ck[:, None] * w[None, 1:1025]

    b = B_EMA
    L = np.zeros((T, T))
    for t in range(T):
        L[t, 1:t + 1] = b * (1 - b) ** (t - np.arange(1, t + 1))
        L[t, 0] = b * (1 - b) ** t + (1 - b) ** (t + 1)
    Pf = np.eye(T)[::-1]
    M = Pf @ L @ Pf @ L

    fpart = np.concatenate([
        np.linspace(0, 1, 33, endpoint=False),
        np.linspace(1, 0, 34),
    ])[1:-1]
    Gf = np.zeros((FP, FP))
    for fo in range(F):
        lo = max(0, fo - 32)
        hi = min(F, fo + 33)
        Gf[lo:hi, fo] = fpart[lo - fo + 32:hi - fo + 32] / 66.0

    npad = NSAMP + NFFT
    norm = np.zeros(npad)
    for t in range(T):
        norm[t * HOP:t * HOP + NFFT] += w**2
    norm = np.where(norm > 1e-10, norm, 1.0)
    rnorm = 1.0 / norm

    # --- device layouts (stationary matrices pre-transposed) ---
    # fwd stationary (128, 8, 1152): [p, a, f] = C*H[f, n], n = 128a+p+1
    WrT = np.zeros((128, 8, FP), np.float16)
    WiT = np.zeros((128, 8, FP), np.float16)
    for a in range(8):
        WrT[:, a, :] = CrH[:, 128 * a:128 * a + 128].T
        WiT[:, a, :] = CiH[:, 128 * a:128 * a + 128].T
    # inverse stationary: [p, kt, n-1] = B*H[128kt+p, n]; Bi kt=8 block is 0
    BrT = np.zeros((128, 9, 1024), np.float16)
    BiT = np.zeros((128, 8, 1024), np.float16)
    for kt in range(9):
        BrT[:, kt, :] = BrH[128 * kt:128 * kt + 128, :]
        if kt < 8:
            BiT[:, kt, :] = BiH[128 * kt:128 * kt + 128, :]
    # EMA stationary (128, 3, 384): [p, st, t'] = M[t', 128st+p]
    MT = np.zeros((128, 3, 384), np.float16)
    for st in range(3):
        s0 = 128 * st
        ns = min(128, T - s0)
        MT[:ns, st, :T] = M[:, s0:s0 + ns].T
    # freq-conv stationary (128, 9, 3, 128)
    GT = np.zeros((128, 9, 3, 128), np.float16)
    for fot in range(9):
        for ix in range(3):
            fit = fot + ix - 1
            if 0 <= fit < 9:
                GT[:, fot, ix, :] = Gf[128 * fit:128 * fit + 128,
                                       128 * fot:128 * fot + 128]
    # OLA reciprocal norm (128, 4, 324): [p, r, j] = rnorm[512j+128r+p]
    RN = np.zeros((128, 4, NCH), np.float32)
    for r in range(4):
        for p in range(128):
            RN[p, r, :] = rnorm[np.arange(NCH) * 512 + 128 * r + p]

    # permutation matrices as lhsT[k_in, m_out] (out[m] = sum_k P[k,m]*in[k])
    def perm(fn, sign=1.0):
        Pm = np.zeros((128, 128), np.float16)
        for m in range(128):
            kk = fn(m)
            if kk is not None:
                Pm[kk, m] = sign
        return Pm
    Jf = perm(lambda m: 127 - m)                         # out[m] = in[127-m]
    S_dn = perm(lambda m: m - 1 if m >= 1 else None)     # out[m] = in[m-1]
    S_cn = perm(lambda m: 127 if m == 0 else None)       # out[0] = in[127]
    PERMS = np.stack([Jf, S_dn, S_cn], axis=1)           # (128, 3, 128)
    return WrT, WiT, BrT, BiT, MT, GT, RN, PERMS


def _register_const(nc, dtype, value):
    t = nc.alloc_sbuf_tensor(f"const-{dtype.name}-{value}", [128, 1], dtype)
    nc.gpsimd.memset(t.ap(), value)
    nc.const_aps.aps[(dtype, value)] = t.ap()


# perm indices in the pm tile
P_JF, P_SDN, P_SCN = range(3)


def _build_nc():
    WrT, WiT, BrT, BiT, MT, GT, RN, PERMS = _build_consts()

    nc = bacc.Bacc("TRN2", target_bir_lowering=False)
    _register_const(nc, f32, -30.0)
    _register_const(nc, f32, 1e-30)
    nc.all_engine_barrier()

    x = nc.dram_tensor("x", [BLOC, NSAMP], f32, kind="ExternalInput")
    y = nc.dram_tensor("y", [BLOC, NSAMP], f32, kind="ExternalOutput")
    dWr = nc.inline_tensor(WrT, name="dWr")
    dWi = nc.inline_tensor(WiT, name="dWi")
    dBr = nc.inline_tensor(BrT, name="dBr")
    dBi = nc.inline_tensor(BiT, name="dBi")
    dMT = nc.inline_tensor(MT, name="dMT")
    dGT = nc.inline_tensor(GT, name="dGT")
    dRN = nc.inline_tensor(RN, name="dRN")
    dPm = nc.inline_tensor(PERMS, name="dPm")
    dI16 = nc.inline_tensor(np.eye(128, dtype=np.float16), name="dI16")

    xv = x.ap().rearrange("b (j c) -> b j c", c=512)   # (4, 320, 512)
    yv = y.ap().rearrange("b (j c) -> b j c", c=512)

    with TileContext(nc) as tc:
        with tc.tile_pool(name="cst", bufs=1) as cp:
            # constant loads ordered/split so the first forward chains are
            # never gated on one big serial DMA: idt16+pm first (framing,
            # fold), then wrh per ft-block in chain order on the sync
            # queue; wih per block on the (startup-idle) ACT queue. The
            # gpsimd queue is left free for the framing xr loads.
            idt16 = cp.tile([128, 128], f16)
            nc.sync.dma_start(out=idt16[:, :], in_=dI16.ap()[:, :])
            pm = cp.tile([128, 3, 128], f16)
            nc.sync.dma_start(out=pm[:, :, :], in_=dPm.ap()[:, :, :])
            wrh = cp.tile([128, 8, FP], f16)
            wih = cp.tile([128, 8, FP], f16)
            # wrh per-ft in chain consumption order [8,0,1,2,6,7,3,4,5];
            # wih rides the startup-idle ACT queue in 3 blocks
            for ft in (8, 0, 1, 2, 6, 7, 3, 4, 5):
                nc.sync.dma_start(out=wrh[:, :, 128 * ft:128 * ft + 128],
                                  in_=dWr.ap()[:, :, 128 * ft:128 * ft + 128])
            for lo, hi in ((0, 384), (768, 1152), (384, 768)):
                nc.scalar.dma_start(out=wih[:, :, lo:hi], in_=dWi.ap()[:, :, lo:hi])
            mt = cp.tile([128, 3, 384], f16)
            nc.sync.dma_start(out=mt[:, :, :], in_=dMT.ap()[:, :, :])
            gt = cp.tile([128, 9, 3, 128], f16)
            nc.sync.dma_start(out=gt[:, :, :, :], in_=dGT.ap()[:, :, :, :])

            with tc.tile_pool(name="sd", bufs=1) as sdp:
                sr = [sdp.tile([128, 9, T], f16, name=f"sr{s}", tag=f"sr{s}")
                      for s in range(BLOC)]
                si = [sdp.tile([128, 8, T], f16, name=f"si{s}", tag=f"si{s}")
                      for s in range(BLOC)]

                winv = sdp.tile([128, 4, NCH], f32, name="rn")
                nc.sync.dma_start(out=winv[:, :, :], in_=dRN.ap()[:, :, :])
                # first inverse-weight pairs preloaded during phase A (the
                # phase-B pool tiles alias phase-A addresses, pinning their
                # DMAs behind the last phase-A readers)
                pre = []
                for a in range(2):
                    brt = sdp.tile([128, 9, 128], f16, name=f"brtp{a}", tag=f"brtp{a}")
                    nc.sync.dma_start(out=brt[:, :, :], in_=dBr.ap()[:, :, 128 * a:128 * a + 128])
                    bit = sdp.tile([128, 8, 128], f16, name=f"bitp{a}", tag=f"bitp{a}")
                    nc.sync.dma_start(out=bit[:, :, :], in_=dBi.ap()[:, :, 128 * a:128 * a + 128])
                    pre.append((brt, bit))
                pools = _phase_a(nc, tc, xv, wrh, wih, pm, mt, gt, idt16, sr, si)
                _phase_b(nc, tc, yv, dBr, dBi, pm, winv, idt16, sr, si, pools, pre)

    nc.finalize()
    return nc


def _phase_a(nc, tc, xv, wrh, wih, pm, mt, gt, idt16, sr, si):
    """Software-pipelined: head(s) = framing..ratio (PE: transposes, fold,
    fwd DFT, EMA), tail(s) = ratio transposes, sigmoid, conv, S*mask.
    tail(s-1) is emitted after head(s). Returns psum pools for phase B."""
    with tc.tile_pool(name="pa", bufs=1) as pa:
        ptp = tc.alloc_tile_pool(name="ptp", bufs=1, space="PSUM")
        pri = tc.alloc_tile_pool(name="pri", bufs=2, space="PSUM")
        ptrp = tc.alloc_tile_pool(name="ptr", bufs=2, space="PSUM")
        pmb = tc.alloc_tile_pool(name="pmb", bufs=3, space="PSUM")

        abs_ = {}
        ratios = {}

        def head1(s):
            # ---- framing: xc[p,mt,j] = xpad[512j+128mt+p] (aligned) and
            #      xc1[p,mt,j] = xpad[512j+128mt+p+1] (+1 shifted) ----
            xc = pa.tile([128, 4, 384], f16, name="xc", tag="xc", bufs=2)
            xc1 = pa.tile([128, 4, 384], f16, name="xc1", tag="xc1", bufs=2)
            for jt in range(3):
                xr = pa.tile([128, 512], f16, name="xr", tag="xr", bufs=2)
                xr1 = pa.tile([128, 512], f16, name="xr1", tag="xr1", bufs=2)
                # xr row p = xpad chunk 128*jt+p; x chunk c lives at xpad
                # chunk c+2. xr1 rows hold samples 1..511 of the chunk in
                # cols 0..510; col 511 (= next chunk's sample 0) is patched
                # once per sample via the xc-row DMA below.
                nc.vector.memset(xr1[:, 511:512], 0.0)
                if jt == 0:
                    nc.vector.memset(xr[0:2, :], 0.0)
                    nc.gpsimd.dma_start(out=xr[2:128, :], in_=xv[s, 0:126, :])
                    nc.vector.memset(xr1[0:2, 0:511], 0.0)
                    nc.gpsimd.dma_start(out=xr1[2:128, 0:511], in_=xv[s, 0:126, 1:512])
                elif jt == 1:
                    nc.gpsimd.dma_start(out=xr[:, :], in_=xv[s, 126:254, :])
                    nc.gpsimd.dma_start(out=xr1[:, 0:511], in_=xv[s, 126:254, 1:512])
                else:
                    nc.vector.memset(xr[64:128, :], 0.0)
                    nc.gpsimd.dma_start(out=xr[0:66, :], in_=xv[s, 254:320, :])
                    nc.vector.memset(xr1[64:128, 0:511], 0.0)
                    nc.gpsimd.dma_start(out=xr1[0:66, 0:511], in_=xv[s, 254:320, 1:512])
                ptg = ptp.tile([128, 4, 128], f16, name="ptg", tag="ptt", bufs=1)
                for mtl in range(4):
                    nc.tensor.transpose(ptg[:, mtl, :], xr[:, 128 * mtl:128 * mtl + 128], idt16[:, :])
                nc.vector.tensor_copy(out=xc[:, :, 128 * jt:128 * jt + 128], in_=ptg[:, :, :])
                ptg1 = ptp.tile([128, 4, 128], f16, name="ptg1", tag="ptt", bufs=1)
                for mtl in range(4):
                    nc.tensor.transpose(ptg1[:, mtl, :], xr1[:, 128 * mtl:128 * mtl + 128], idt16[:, :])
                nc.vector.tensor_copy(out=xc1[:, :, 128 * jt:128 * jt + 128], in_=ptg1[:, :, :])
            # patch xc1[127, 3, j] = xpad[512(j+1)] = xc[0, 0, j+1]
            nc.gpsimd.dma_start(out=xc1[127:128, 3, 0:322], in_=xc[0:1, 0, 1:323])

            # ---- fold: pu_a = flip(xc_af); u = xc1_a + pu, v = xc1_a - pu ----
            ut = pa.tile([128, 8, T], f16, name="ut", tag="ut", bufs=2)
            vt = pa.tile([128, 8, T], f16, name="vt", tag="vt", bufs=2)
            for a in range(8):
                af = 15 - a
                pu = pmb.tile([128, T], f32, name="pu", tag="pmb", bufs=3)
                nc.tensor.matmul(pu[:, :], pm[:, P_JF, :], xc[:, af % 4, af // 4:af // 4 + T],
                                 start=True, stop=True)
                puS = pa.tile([128, T], f16, name="puS", tag="puS", bufs=3)
                nc.scalar.copy(out=puS[:, :], in_=pu[:, :])
                nc.vector.tensor_tensor(out=ut[:, a, :], in0=xc1[:, a % 4, a // 4:a // 4 + T],
                                        in1=puS[:, :], op=OP.add)
                nc.gpsimd.tensor_tensor(out=vt[:, a, :], in0=xc1[:, a % 4, a // 4:a // 4 + T],
                                        in1=puS[:, :], op=OP.subtract)

            # ---- forward DFT (half-range contraction) + |S| ----
            # Chain order [8,0,1,2,6,7,3,4,5]: the |S|-transpose groups
            # consume ab in fg order [0,2,1], so groups fg=0 (ft 0-2) and
            # fg=2 (ft 6-8) are ready while fg=1 (ft 3-5) finishes last.
            # ft=8 (Nyquist) has no imag chain (sin(pi*n) = 0).
            ab = {}
            for ft in [8, 0, 1, 2, 6, 7, 3, 4, 5]:
                pr = pri.tile([128, T], f32, name="pr", tag="pri", bufs=2)
                for a in range(8):
                    nc.tensor.matmul(pr[:, :], wrh[:, a, 128 * ft:128 * ft + 128],
                                     ut[:, a, :], start=(a == 0), stop=(a == 7))
                nc.scalar.copy(out=sr[s][:, ft, :], in_=pr[:, :])
                sq = pa.tile([128, T], f16, name="sq", tag="sq", bufs=2)
                nc.vector.tensor_tensor(out=sq[:, :], in0=sr[s][:, ft, :], in1=sr[s][:, ft, :], op=OP.mult)
                if ft < 8:
                    pi = pri.tile([128, T], f32, name="pi", tag="pri", bufs=2)
                    for a in range(8):
                        nc.tensor.matmul(pi[:, :], wih[:, a, 128 * ft:128 * ft + 128],
                                         vt[:, a, :], start=(a == 0), stop=(a == 7))
                    nc.scalar.copy(out=si[s][:, ft, :], in_=pi[:, :])
                    sq2 = pa.tile([128, T], f16, name="sq2", tag="sq2", bufs=2)
                    nc.gpsimd.tensor_tensor(out=sq2[:, :], in0=si[s][:, ft, :], in1=si[s][:, ft, :], op=OP.mult)
                    nc.vector.tensor_tensor(out=sq[:, :], in0=sq[:, :], in1=sq2[:, :], op=OP.add)
                abt = pa.tile([128, T], f16, name="ab", tag=f"ab{ft}", bufs=1)
                nc.scalar.sqrt(out=abt[:, :], in_=sq[:, :])
                ab[ft] = abt
            abs_[s] = ab

        def head2(s):
            ab = abs_[s]
            # ---- transpose |S| -> absT (t, f) ----
            at = pa.tile([128, 3, FP], f16, name="absT", tag="absT", bufs=1)
            nc.gpsimd.memset(at[64:128, 2, :], 0.0)
            for tt in range(3):
                cols = 128 if tt < 2 else T - 256
                for fg in [0, 2, 1]:
                    ptg = ptp.tile([128, 3, 128], f16, name="ptg2", tag="ptt", bufs=1)
                    for i in range(3):
                        ft = 3 * fg + i
                        nc.tensor.transpose(ptg[0:cols, i, :], ab[ft][:, 128 * tt:128 * tt + cols], idt16[:, :])
                    nc.vector.tensor_copy(out=at[0:cols, tt, 384 * fg:384 * fg + 384], in_=ptg[0:cols, :, :])
            # padded bins (f > 1024) -> 1.0 so the EMA reciprocal stays
            # finite without a bias stage; their sigmoid(10*1-30) ~ 0
            nc.vector.memset(at[:, :, F:FP], 1.0)

            # ---- EMA smooth (matmuls; Pool evac +1e-30) ----
            r0t3 = pa.tile([128, 3, FP], f32, name="r0t3", tag="r0t3", bufs=1)
            for tt in range(3):
                for fc in [0, 2, 1]:
                    pe = pmb.tile([128, 384], f32, name="pema", tag="pmb", bufs=3)
                    for st in range(3):
                        nc.tensor.matmul(pe[:, :], mt[:, st, 128 * tt:128 * tt + 128],
                                         at[:, st, 384 * fc:384 * fc + 384],
                                         start=(st == 0), stop=(st == 2))
                    nc.vector.reciprocal_approx_fast(out=r0t3[:, tt, 384 * fc:384 * fc + 384],
                                                     in_=pe[:, :])
            # ---- ratio (fp16); sigmoid below uses scale=+10 ----
            ratio = pa.tile([128, 3, FP], f16, name="ratio", tag="ratio", bufs=2)
            for tt in range(3):
                nc.gpsimd.tensor_tensor(out=ratio[:, tt, :], in0=at[:, tt, :],
                                        in1=r0t3[:, tt, :], op=OP.mult)
            ratios[s] = ratio

        def tail(s):
            ratio = ratios[s]
            # ---- per-ft: transpose -ratio -> (f, t), sigmoid(-10x-30),
            # 3-tap time conv; the banded freq conv for fot is emitted as
            # soon as mtc[fot+1] exists so the PE never waits a full
            # sigmoid sweep. ----
            msk = pa.tile([128, 9, T + 2], f16, name="msk", tag="msk", bufs=1)
            mtc = pa.tile([128, 9, T], f16, name="mtc", tag="mtc", bufs=1)
            nc.vector.memset(msk[:, :, 0:1], 0.0)
            nc.vector.memset(msk[:, :, T + 1:T + 2], 0.0)

            def conv(fot):
                pmn = pmb.tile([128, T], f32, name="pmsm", tag="pmb", bufs=3)
                ixs = [ix for ix in range(3) if 0 <= fot + ix - 1 < 9]
                for i, ix in enumerate(ixs):
                    nc.tensor.matmul(pmn[:, :], gt[:, fot, ix, :], mtc[:, fot + ix - 1, :],
                                     start=(i == 0), stop=(i == len(ixs) - 1))
                pmnS = pa.tile([128, T], f16, name="pmnS", tag="pmnS", bufs=2)
                nc.scalar.copy(out=pmnS[:, :], in_=pmn[:, :])
                nc.vector.tensor_tensor(out=sr[s][:, fot, :], in0=pmnS[:, :], in1=sr[s][:, fot, :], op=OP.mult)
                if fot < 8:
                    nc.gpsimd.tensor_tensor(out=si[s][:, fot, :], in0=pmnS[:, :], in1=si[s][:, fot, :], op=OP.mult)

            for ft in range(9):
                ptr = ptrp.tile([128, 384], f16, name="ptr", tag="ptr", bufs=2)
                for tt in range(3):
                    nc.tensor.transpose(ptr[:, 128 * tt:128 * tt + 128],
                                        ratio[:, tt, 128 * ft:128 * ft + 128], idt16[:, :])
                nc.scalar.activation(out=msk[:, ft, 1:1 + T], in_=ptr[:, 0:T],
                                     func=AF.Sigmoid, scale=10.0, bias=-30.0)
                nc.gpsimd.tensor_tensor(out=mtc[:, ft, :], in0=msk[:, ft, 0:T],
                                        in1=msk[:, ft, 2:T + 2], op=OP.add)
                nc.vector.scalar_tensor_tensor(out=mtc[:, ft, :], in0=mtc[:, ft, :], scalar=0.5,
                                               in1=msk[:, ft, 1:T + 1], op0=OP.mult, op1=OP.add)
                if ft >= 1:
                    conv(ft - 1)
            conv(8)

        # two-deep software pipeline: each sample's |S|/EMA/gate elementwise
        # chain hides under the NEXT sample's forward-DFT matmul stretch
        head1(0)
        head1(1)
        head2(0)
        head1(2)
        head2(1)
        tail(0)
        head1(3)
        head2(2)
        tail(1)
        head2(3)
        tail(2)
        tail(3)
        return ptp, pri, ptrp, pmb


def _phase_b(nc, tc, yv, dBr, dBi, pm, rn, idt16, sr, si, pools, pre):
    """Half-range inverse DFT, combine + overlap-add in PSUM, normalize,
    fp16 transpose, store."""
    ptp, pri, ptrp, pmb = pools
    # phase A psum pools are done; rebuild the bank budget for phase B:
    # 3-deep inverse chain rotation + 4 OLA accumulators + transpose slot
    for p in (pmb, ptrp, pri, ptp):
        p.release()
    pinv = tc.alloc_tile_pool(name="pinv", bufs=2, space="PSUM")
    pacc = tc.alloc_tile_pool(name="pacc", bufs=1, space="PSUM")
    ptob = tc.alloc_tile_pool(name="ptob", bufs=1, space="PSUM")
    with tc.tile_pool(name="pb", bufs=1) as pb:
        # a=0,1 were preloaded during phase A; load the rest now
        brt = [pre[0][0], pre[1][0]] + [pb.tile([128, 9, 128], f16, name=f"brt{a}", tag=f"brt{a}")
                                        for a in range(2, 8)]
        bit = [pre[0][1], pre[1][1]] + [pb.tile([128, 8, 128], f16, name=f"bit{a}", tag=f"bit{a}")
                                        for a in range(2, 8)]
        for a in range(2, 8):
            nc.sync.dma_start(out=brt[a][:, :, :], in_=dBr.ap()[:, :, 128 * a:128 * a + 128])
            nc.gpsimd.dma_start(out=bit[a][:, :, :], in_=dBi.ap()[:, :, 128 * a:128 * a + 128])
        # sample-outer: each sample's combine/store overlaps the next
        # sample's inverse chains on the PE queue
        for s in range(BLOC):
            yp = pb.tile([128, 8, T], f16, name="yp", tag="yp", bufs=2)
            ym = pb.tile([128, 8, T], f16, name="ym", tag="ym", bufs=2)
            for a in range(8):
                pp = pinv.tile([128, T], f32, name="pp", tag="pri", bufs=2)
                for kt in range(9):
                    nc.tensor.matmul(pp[:, :], brt[a][:, kt, :], sr[s][:, kt, :],
                                     start=(kt == 0), stop=(kt == 8))
                pq = pinv.tile([128, T], f32, name="pq", tag="pri", bufs=2)
                for kt in range(8):
                    nc.tensor.matmul(pq[:, :], bit[a][:, kt, :], si[s][:, kt, :],
                                     start=(kt == 0), stop=(kt == 7))
                ppS = pb.tile([128, T], f16, name="ppS", tag="ppS", bufs=2)
                nc.scalar.copy(out=ppS[:, :], in_=pp[:, :])
                nc.vector.tensor_tensor(out=yp[:, a, :], in0=ppS[:, :], in1=pq[:, :], op=OP.add)
                # ym = pp - pq = 2*ppS - yp: keeps pq's PSUM slot free for
                # the next chain one evacuation earlier
                nc.vector.scalar_tensor_tensor(out=ym[:, a, :], in0=ppS[:, :], scalar=2.0,
                                               in1=yp[:, a, :], op0=OP.mult, op1=OP.subtract)
            # combine into chunk layout with OLA accumulated directly in PSUM
            acc = {}
            for rr in range(4):
                a4 = pacc.tile([128, NCH], f32, name="acc", tag=f"acc{rr}", bufs=1)
                acc[rr] = a4
                nc.vector.memset(a4[:, :], 0.0)
            for nt in range(16):
                d, rrr = nt // 4, nt % 4
                tgt = acc[rrr][:, d:d + T]
                if nt <= 7:
                    # y[128nt+p] = yp[a=nt][p-1]; p=0 row from yp[a=nt-1][127]
                    nc.tensor.matmul(tgt, pm[:, P_SDN, :], yp[:, nt, :],
                                     start=False, stop=(nt == 0), skip_group_check=True)
                    if nt > 0:
                        nc.tensor.matmul(tgt, pm[:, P_SCN, :], yp[:, nt - 1, :],
                                         start=False, stop=True, skip_group_check=True)
                else:
                    # y[128nt+p] = ym[a=15-nt][127-p]  (p=0: n=1024, Q=0 there)
                    nc.tensor.matmul(tgt, pm[:, P_JF, :], ym[:, 15 - nt, :],
                                     start=False, stop=True, skip_group_check=True)
            for rr in range(4):
                a16 = pb.tile([128, NCH], f16, name="a16", tag=f"a16{rr}", bufs=2)
                nc.vector.tensor_tensor(out=a16[:, :], in0=acc[rr][:, :], in1=rn[:, rr, :], op=OP.mult)
                for jt in range(3):
                    cj = 128 if jt < 2 else NCH - 256
                    r0 = max(2, 128 * jt) - 128 * jt
                    r1 = min(322, 128 * jt + cj) - 128 * jt
                    pt = ptob.tile([128, 128], f16, name="pto", tag="pto", bufs=1)
                    nc.tensor.transpose(pt[0:cj, :], a16[:, 128 * jt:128 * jt + cj], idt16[:, :])
                    ob = pb.tile([128, 128], f32, name="ob", tag="ob", bufs=8)
                    nc.scalar.copy(out=ob[0:cj, :], in_=pt[0:cj, :])
                    nc.sync.dma_start(out=yv[s, 128 * jt + r0 - 2:128 * jt + r1 - 2, 128 * rr:128 * rr + 128],
                                      in_=ob[r0:r1, :])
    ptob.release()
    pacc.release()
    pinv.release()


_NC = None


def _get_nc():
    global _NC
    if _NC is None:
        _NC = _build_nc()
    return _NC


def _run(x, trace=False):
    nc = _get_nc()
    x = np.ascontiguousarray(np.asarray(x), np.float32)
    assert x.shape == (NCORES * BLOC, NSAMP)
    in_maps = [{"x": x[BLOC * i:BLOC * i + BLOC]} for i in range(NCORES)]
    res = run_bass_kernel_spmd(nc, in_maps, list(range(NCORES)), trace=trace)
    out = np.concatenate([res.results[i]["y"] for i in range(NCORES)], axis=0)
    return out, res


def kernel(x):
    out, _ = _run(x)
    return out
